# revision 1
# baseline (speedup 1.0000x reference)
"""Trainium2 Bass kernel for NeuronGemma4VisionAttention.

Problem: B=2, P=4096, HID=1152, 16 heads x 72 dim, fp32.
  q,k,v = x@Wq, x@Wk, x@Wv  -> per-head RMSNorm (q,k learned scale, v none)
  -> 2-part RoPE on q,k -> softmax(q k^T) v -> concat heads @ Wo

Sharding (8 cores, one chip):
  Head-parallel: core c owns heads (2c, 2c+1) for BOTH batches.
  Each core: QKV projection (its 144 columns of each W), per-head norm+rope,
  full non-causal attention for its 2 heads x 2 batches, then an 8-core
  AllToAll exchanges token-eighths so core c ends with the full 1152-dim
  attention output for tokens [1024*(c%4) ... ) of batch c//4, on which it
  runs the o_proj. Host reassembles the 8 [1152, 1024] output slices.

Numerics:
  - All matmuls in float32r (TF32-like, ~11-bit mantissa): measured e2e
    rel err ~1.4e-3 vs fp32 reference.
  - Softmax stability: subtract c_q = 8*|q|_2 per query token, folded into
    the scores matmul via an augmented contraction row (row 72 of K^T is
    ones, row 72 of Q^T is -c_q). Empirically max(rowmax-c)=57 < 80 and
    max(c-rowmax)=61 < 85, so exp never overflows/underflows.
  - Softmax denominator: ones column appended to V (col 72) makes row 72 of
    the PV product the per-query sum of exp.
  - ACT uses only Copy/Ln/Exp -> single activation table, no reloads.
"""
import os
import sys

sys.path.insert(0, "/opt/trn_rl_repo")

import numpy as np

import concourse.bass as bass  # noqa: F401
import concourse.tile as tile
from concourse import bacc, mybir
from concourse.bass_utils import run_bass_kernel_spmd
from concourse.masks import make_identity

F32 = mybir.dt.float32
F32R = mybir.dt.float32r
AF = mybir.ActivationFunctionType

N_CORES = 8
B, P, HID = 2, 4096, 1152
NH, D = 16, 72
HL = 2                # heads per core
TB = B * P            # 8192 tokens across batches
NBLK = TB // 128      # 64 token blocks
KBLK = P // 128       # 32 key blocks per batch
QC = 512              # query chunk
NQC = P // QC         # 8 query chunks per batch
BETA = 8.0
EPS = 1e-6

_CACHED_NC = None


def _build_nc():
    nc = bacc.Bacc("TRN2", target_bir_lowering=False, debug=False,
                   num_devices=N_CORES)

    xT = nc.dram_tensor("xT", [HID, TB], F32R, kind="ExternalInput").ap()
    wqkv = nc.dram_tensor("wqkv", [HID, 3 * HL * D], F32R,
                          kind="ExternalInput").ap()
    # ropec rows per token: [cwq, swq, cwk, swk] each [72]
    ropec = nc.dram_tensor("ropec", [TB, 4, D], F32, kind="ExternalInput").ap()
    wo = nc.dram_tensor("wo", [HID, HID], F32R, kind="ExternalInput").ap()
    outT = nc.dram_tensor("outT", [HID, 1024], F32, kind="ExternalOutput").ap()

    xT_v = xT.rearrange("(c p) t -> p c t", p=128)       # [128, 9, 8192]
    wqkv_v = wqkv.rearrange("(c p) n -> p c n", p=128)   # [128, 9, 432]
    wo_v = wo.rearrange("(c p) n -> p c n", p=128)       # [128, 9, 1152]

    with tile.TileContext(nc) as tc:
        with (
            tc.tile_pool(name="persist", bufs=1) as persist,
            tc.tile_pool(name="dram", bufs=1, space="DRAM") as dram,
        ):
            # ---- persistent state ----
            ident = persist.tile([128, 128], F32, tag="ident")
            make_identity(nc, ident)
            epst = persist.tile([128, 1], F32, tag="epst")
            nc.vector.memset(epst[:], EPS)
            eps0 = persist.tile([128, 1], F32, tag="eps0")
            nc.vector.memset(eps0[:], 1e-20)
            qt_dram = {}
            for b in range(B):
                for hl in range(HL):
                    qt_dram[(b, hl)] = dram.tile([73, P], F32R,
                                                 name=f"qtd_{b}_{hl}")
            a2a_in = dram.tile([N_CORES, HL * D, 1024], F32R)
            a2a_out = dram.tile([N_CORES, HL * D, 1024], F32R)

            # ============ attention state (freed before o_proj) ============
            astate_cm = tc.tile_pool(name="astate", bufs=1)
            astate = astate_cm.__enter__()
            kt = {}
            for b in range(B):
                for hl in range(HL):
                    kt[(b, hl)] = astate.tile([73, P], F32R,
                                              name=f"kt_{b}_{hl}",
                                              tag=f"kt_{b}_{hl}")
            # V padded to 97 cols: ones at col 96 (partition-base-aligned
            # row 96 of the PV psum holds the softmax denominators)
            vaug = [astate.tile([128, KBLK, HL, 97], F32R,
                                name=f"vaug_{b}", tag=f"vaug_{b}")
                    for b in range(B)]
            wqkv_sb = astate.tile([128, 9, 3 * HL * D], F32R, tag="wqkv")
            nc.sync.dma_start(wqkv_sb[:], wqkv_v)
            for b in range(B):
                nc.vector.memset(vaug[b][:].bitcast(F32), 0.0)
                nc.vector.memset(vaug[b][:, :, :, 96].bitcast(F32), 1.0)

            # ================= Phase 1: QKV + norm + rope =================
            with (
                tc.tile_pool(name="p1", bufs=3) as p1,
                tc.tile_pool(name="p1ps", bufs=2, space="PSUM") as p1ps,
                tc.tile_pool(name="trps", bufs=4, space="PSUM") as trps,
            ):
                for blk in range(NBLK):
                    b, kb = blk // KBLK, blk % KBLK
                    tsl = slice(blk * 128, (blk + 1) * 128)

                    xt_t = p1.tile([128, 9, 128], F32R, tag="xt")
                    nc.sync.dma_start(xt_t[:], xT_v[:, :, tsl])
                    rc = p1.tile([128, 4, D], F32, tag="rc")
                    nc.sync.dma_start(rc[:], ropec[tsl])

                    ps = p1ps.tile([128, 3 * HL * D], F32, tag="psqkv")
                    for c in range(9):
                        nc.tensor.matmul(ps[:], xt_t[:, c, :],
                                         wqkv_sb[:, c, :],
                                         start=(c == 0), stop=(c == 8))

                    sb = p1.tile([128, 3 * HL * D], F32, tag="qkv")
                    nc.scalar.activation(sb[:], ps[:], AF.Copy)

                    # raw sum-of-squares for the 6 (tensor, head) groups
                    sq = p1.tile([128, 3 * HL * D], F32, tag="sq")
                    nc.vector.tensor_mul(sq[:], sb[:], sb[:])
                    ssr = p1.tile([128, 3 * HL], F32, tag="ssr")
                    nc.vector.reduce_sum(ssr[:],
                                         sq[:].rearrange("p (g d) -> p g d",
                                                         d=D),
                                         axis=mybir.AxisListType.X)
                    # alpha = (ssr/72 + eps)^-1/2 = exp(-0.5*ln(...))
                    al = p1.tile([128, 3 * HL], F32, tag="al")
                    nc.scalar.activation(al[:], ssr[:], AF.Ln,
                                         bias=epst[:], scale=1.0 / D)
                    nc.scalar.activation(al[:], al[:], AF.Exp, scale=-0.5)

                    # rope on q (groups 0:2) and k (groups 2:4)
                    qk5 = sb[:, 0:2 * HL * D].rearrange(
                        "p (g a c j) -> p g a c j", g=2 * HL, a=2, c=2)
                    rc5 = rc[:].rearrange("p r (a c j) -> p r a c j",
                                          a=2, c=2)
                    rp = p1.tile([128, 2 * HL, D], F32, tag="rope")
                    rp5 = rp[:].rearrange("p g (a c j) -> p g a c j",
                                          a=2, c=2)
                    tmp = p1.tile([128, HL, 2, 18], F32, tag="ropetmp")
                    shp = [128, HL, 2, 18]
                    for t in range(2):  # 0: q, 1: k
                        gs = slice(t * HL, (t + 1) * HL)
                        cw0 = rc5[:, 2 * t, :, 0, :].unsqueeze(
                            1).to_broadcast(shp)
                        cw1 = rc5[:, 2 * t, :, 1, :].unsqueeze(
                            1).to_broadcast(shp)
                        sw0 = rc5[:, 2 * t + 1, :, 0, :].unsqueeze(
                            1).to_broadcast(shp)
                        sw1 = rc5[:, 2 * t + 1, :, 1, :].unsqueeze(
                            1).to_broadcast(shp)
                        x = qk5[:, gs]
                        r = rp5[:, gs]
                        nc.vector.tensor_mul(r[:, :, :, 0, :],
                                             x[:, :, :, 0, :], cw0)
                        nc.vector.tensor_mul(tmp[:], x[:, :, :, 1, :], sw0)
                        nc.vector.tensor_sub(r[:, :, :, 0, :],
                                             r[:, :, :, 0, :], tmp[:])
                        nc.vector.tensor_mul(r[:, :, :, 1, :],
                                             x[:, :, :, 1, :], cw1)
                        nc.vector.tensor_mul(tmp[:], x[:, :, :, 0, :], sw1)
                        nc.vector.tensor_add(r[:, :, :, 1, :],
                                             r[:, :, :, 1, :], tmp[:])

                    # q_aug / k_aug token-major [128, HL, 73]
                    qaug = p1.tile([128, HL, 73], F32, tag="qaug")
                    kaug = p1.tile([128, HL, 73], F32, tag="kaug")
                    nc.vector.tensor_mul(
                        qaug[:, :, 0:D], rp[:, 0:HL, :],
                        al[:, 0:HL].unsqueeze(2).to_broadcast([128, HL, D]))
                    nc.vector.tensor_mul(
                        kaug[:, :, 0:D], rp[:, HL:2 * HL, :],
                        al[:, HL:2 * HL].unsqueeze(2).to_broadcast(
                            [128, HL, D]))
                    nc.vector.memset(kaug[:, :, D], 1.0)

                    # c_q = 8 * alpha_q * |rope(q_raw)|
                    sqq = p1.tile([128, HL, D], F32, tag="sqq")
                    nc.vector.tensor_mul(sqq[:], rp[:, 0:HL, :],
                                         rp[:, 0:HL, :])
                    ss2 = p1.tile([128, HL], F32, tag="ss2")
                    nc.vector.reduce_sum(ss2[:], sqq[:],
                                         axis=mybir.AxisListType.X)
                    cqt = p1.tile([128, HL], F32, tag="cqt")
                    nc.scalar.activation(cqt[:], ss2[:], AF.Ln,
                                         bias=eps0[:], scale=1.0)
                    nc.scalar.activation(cqt[:], cqt[:], AF.Exp, scale=0.5)
                    nc.vector.tensor_mul(cqt[:], cqt[:], al[:, 0:HL])
                    nc.scalar.activation(qaug[:, :, D], cqt[:], AF.Copy,
                                         scale=-BETA)

                    # v with norm into persistent vaug
                    nc.vector.tensor_mul(
                        vaug[b][:, kb, :, 0:D],
                        sb[:].rearrange("p (g d) -> p g d", d=D)[:,
                                                                2 * HL:3 * HL,
                                                                :],
                        al[:, 2 * HL:3 * HL].unsqueeze(2).to_broadcast(
                            [128, HL, D]))

                    # transpose q/k to feature-major
                    ksl = slice(kb * 128, (kb + 1) * 128)
                    for hl in range(HL):
                        tq = trps.tile([73, 128], F32, tag="tr", name="tq")
                        nc.tensor.transpose(tq[:], qaug[:, hl, :], ident[:])
                        qs = p1.tile([73, 128], F32R, tag="qs")
                        nc.scalar.activation(qs[:], tq[:], AF.Copy)
                        nc.sync.dma_start(qt_dram[(b, hl)][:, ksl], qs[:])
                        tk = trps.tile([73, 128], F32, tag="tr", name="tk")
                        nc.tensor.transpose(tk[:], kaug[:, hl, :], ident[:])
                        nc.scalar.activation(kt[(b, hl)][:, ksl], tk[:],
                                             AF.Copy)

            # ================= Phase 2: attention =================
            with (
                tc.tile_pool(name="p2", bufs=4) as p2,
                tc.tile_pool(name="p2sm", bufs=3) as p2sm,
                tc.tile_pool(name="p2o", bufs=2, space="PSUM") as p2o,
                tc.tile_pool(name="p2s", bufs=3, space="PSUM") as p2s,
            ):
                for b in range(B):
                    for hl in range(HL):
                        for qc in range(NQC):
                            qsl = slice(qc * QC, (qc + 1) * QC)
                            qt_t = p2.tile([73, QC], F32R, tag="qt")
                            nc.sync.dma_start(qt_t[:],
                                              qt_dram[(b, hl)][:, qsl])
                            pso = p2o.tile([97, QC], F32, tag="pso")
                            for kb in range(KBLK):
                                ksl = slice(kb * 128, (kb + 1) * 128)
                                pss = p2s.tile([128, QC], F32, tag="pss")
                                nc.tensor.matmul(pss[:],
                                                 kt[(b, hl)][:, ksl],
                                                 qt_t[:],
                                                 start=True, stop=True)
                                pt = p2.tile([128, QC], F32R, tag="pt")
                                nc.scalar.activation(pt[:], pss[:], AF.Exp)
                                nc.tensor.matmul(pso[:],
                                                 vaug[b][:, kb, hl, :],
                                                 pt[:],
                                                 start=(kb == 0),
                                                 stop=(kb == KBLK - 1))
                            rec = p2sm.tile([1, QC], F32, tag="rec")
                            nc.vector.reciprocal(rec[:], pso[96:97, :])
                            bct = p2sm.tile([D, QC], F32, tag="bct")
                            nc.gpsimd.partition_broadcast(bct[:], rec[:])
                            onrm = p2sm.tile([D, QC], F32R, tag="onrm")
                            nc.vector.tensor_mul(onrm[:], pso[0:D, :],
                                                 bct[:])
                            e = b * 4 + qc // 2
                            csl = slice((qc % 2) * QC, (qc % 2) * QC + QC)
                            nc.sync.dma_start(
                                a2a_in[e, hl * D:(hl + 1) * D, csl],
                                onrm[:])

            astate_cm.__exit__(None, None, None)

            # ================= Phase 3: A2A + o_proj =================
            nc.gpsimd.collective_compute(
                "AllToAll", mybir.AluOpType.bypass,
                ins=[a2a_in[:]], outs=[a2a_out[:]],
                replica_groups=[list(range(N_CORES))],
            )
            with (
                tc.tile_pool(name="p3", bufs=1) as p3,
                tc.tile_pool(name="p3o", bufs=2) as p3o,
                tc.tile_pool(name="p3ps", bufs=4, space="PSUM") as p3ps,
            ):
                yt = p3.tile([128, 9, 1024], F32R, tag="yt")
                nc.sync.dma_start(
                    yt[:],
                    a2a_out[:].rearrange("j r t -> (j r) t").rearrange(
                        "(c p) t -> p c t", p=128))
                wo_sb = p3.tile([128, 9, HID], F32R, tag="wo")
                nc.sync.dma_start(wo_sb[:], wo_v)
                for fo in range(9):
                    for tcn in range(2):
                        ps3 = p3ps.tile([128, QC], F32, tag="ps3")
                        for fi in range(9):
                            nc.tensor.matmul(
                                ps3[:],
                                wo_sb[:, fi, fo * 128:(fo + 1) * 128],
                                yt[:, fi, tcn * QC:(tcn + 1) * QC],
                                start=(fi == 0), stop=(fi == 8))
                        ot = p3o.tile([128, QC], F32, tag="ot")
                        nc.vector.tensor_copy(ot[:], ps3[:])
                        nc.sync.dma_start(
                            outT[fo * 128:(fo + 1) * 128,
                                 tcn * QC:(tcn + 1) * QC], ot[:])

    nc.compile()
    return nc


def _prep_inputs(inputs):
    hs = np.ascontiguousarray(np.asarray(inputs["hidden_states"],
                                         dtype=np.float32))
    cos = np.asarray(inputs["cos"], dtype=np.float32)
    sin = np.asarray(inputs["sin"], dtype=np.float32)
    Wq = np.asarray(inputs["Wq"], dtype=np.float32)
    Wk = np.asarray(inputs["Wk"], dtype=np.float32)
    Wv = np.asarray(inputs["Wv"], dtype=np.float32)
    Wo = np.ascontiguousarray(np.asarray(inputs["Wo"], dtype=np.float32))
    qw = np.asarray(inputs["q_norm_w"], dtype=np.float32)
    kw = np.asarray(inputs["k_norm_w"], dtype=np.float32)

    xT = np.ascontiguousarray(hs.transpose(2, 0, 1).reshape(HID, TB))

    # partner index for the sin term of 2-part rope
    partner = np.empty(D, np.int64)
    for a in range(2):
        base = a * 36
        partner[base:base + 18] = np.arange(base + 18, base + 36)
        partner[base + 18:base + 36] = np.arange(base, base + 18)
    cs = cos.reshape(TB, D)
    sn = sin.reshape(TB, D)
    ropec = np.stack([cs * qw[None, :], sn * qw[partner][None, :],
                      cs * kw[None, :], sn * kw[partner][None, :]],
                     axis=1)
    ropec = np.ascontiguousarray(ropec.astype(np.float32))

    in_maps = []
    for c in range(N_CORES):
        colsl = slice(c * HL * D, (c + 1) * HL * D)
        wqkv = np.ascontiguousarray(
            np.concatenate([Wq[:, colsl], Wk[:, colsl], Wv[:, colsl]],
                           axis=1))
        in_maps.append({
            "xT": xT,
            "wqkv": wqkv,
            "ropec": ropec,
            "wo": Wo,
        })
    return in_maps


def kernel(**inputs):
    global _CACHED_NC
    if _CACHED_NC is None:
        _CACHED_NC = _build_nc()
    nc = _CACHED_NC
    in_maps = _prep_inputs(inputs)
    trace = bool(int(os.environ.get("KERNEL_TRACE", "0")))
    res = run_bass_kernel_spmd(nc, in_maps, core_ids=list(range(N_CORES)),
                               trace=trace)
    kernel.last_results = res
    out = np.empty((B, P, HID), dtype=np.float32)
    for c in range(N_CORES):
        b, qtr = c // 4, c % 4
        out[b, qtr * 1024:(qtr + 1) * 1024, :] = \
            res.results[c]["outT"].T
    return out

